# revision 50
# baseline (speedup 1.0000x reference)
"""Causal multi-head attention (B=4, S=2048, H=16, D=64, E=1024) on 8 TRN2 cores.

Sharding: data-parallel over batch (4) x tensor-parallel over heads (2 groups
of 8). Each core computes, for its (batch, head-group):
    q/k/v projections -> causal softmax attention -> output projection
and returns a partial [S, E] output (Wr row-split); the host adds the two
partials per batch.

Engine plan (per core):
  - Inputs (x, Wq, Wk, Wv, Wr) are host-cast to bf16: halves the DMA and
    runs the projection matmuls at 1 cycle/row.  qt/kt are bf16 too (f32r
    weights disable the fast weight-load path and ran ~2.3x slower).
  - Scores per head pair run as two row-tiled concurrent matmuls (head-even
    on PE rows 0-63, head-odd on rows 64-127, outputs in different PSUM
    banks), so the K=64 contraction doesn't waste half the array.
  - Diagonal key-blocks are processed FIRST with their fully-masked left
    query columns skipped entirely (scores, exp and attn@V all shrink by
    ~15%); only the 128-wide diagonal sub-block needs the tri mask.
  - exp() on ScalarE over one [128, 2, 512] ACTIVATE per key block (ACT has
    ~300ns fixed overhead per instruction - do not split it).
  - attn@V with a ones-column on V so the softmax denominators fall out of
    the same matmul (output row 64); V rows padded to 128 so its weight
    loads take the FWL path.  Denominators inverted with the fast
    custom-DVE reciprocal and broadcast to 64 rows via a tiny PE matmul.
  - The attention loop is software-pipelined 2 deep (scores run two
    iterations ahead of attn@V) and projection / output-projection work is
    woven in as WHOLE units of filler (finer slicing entangles the
    scheduler and regresses) so the PE never stalls on ScalarE exp and the
    HAM clock gate stays warm.
  - Startup: memset-fed warmup matmuls from t~1us, DMA ordered so the
    first chunk's q/k/v projections start as soon as their bytes land.
"""

from collections import deque

import numpy as np

import concourse.bacc as bacc
import concourse.bass as bass
import concourse.mybir as mybir
import concourse.tile as tile
from concourse.bass_utils import run_bass_kernel_spmd
from concourse.dve_ops import RECIP_APPROX_FAST_CONSTS, RECIPROCAL_APPROX_FAST

HEADS = 16
HD = 64
EMB = 1024
B, S = 4, 2048
SCALE = 1.0 / 8.0
NCORES = 8
HPC = HEADS // 2          # heads per core (8)
GW = HPC * HD             # head-group width (512)

F32 = mybir.dt.float32
F32R = mybir.dt.float32r
BF16 = mybir.dt.bfloat16
FP8 = mybir.dt.float8e4
DR = mybir.MatmulPerfMode.DoubleRow
EXP = mybir.ActivationFunctionType.Exp

FP8_P2 = False             # q/k projections in fp8e4m3 DoubleRow
WSC = 64.0                # host pre-scale on Wq/Wk so fp8 hits its sweet range
ESC = SCALE / (WSC * WSC) if FP8_P2 else 1.0  # folded into exp()'s input scale

NQC = 4                   # query chunks of 512
QW = 512                  # query chunk width
NEC = EMB // 128          # emb chunks (8)
NSB = S // 128            # seq blocks (16)


def build():
    nc = bacc.Bacc("TRN2", target_bir_lowering=False, debug=False)

    WDT = FP8 if FP8_P2 else BF16
    xt_d = nc.dram_tensor("xt", [EMB, S], BF16, kind="ExternalInput")
    # weights pre-swizzled on host to partition-major so every DMA is one
    # contiguous transfer
    wq_d = nc.dram_tensor("wq", [128, 4, NEC, 128], WDT, kind="ExternalInput")
    wk_d = nc.dram_tensor("wk", [128, 4, NEC, 128], WDT, kind="ExternalInput")
    wv_d = nc.dram_tensor("wv", [128, NEC, GW], BF16, kind="ExternalInput")
    wr_d = nc.dram_tensor("wr", [128, 4, EMB], BF16, kind="ExternalInput")
    if FP8_P2:
        xt8_d = nc.dram_tensor("xt8", [128, NEC, S], FP8, kind="ExternalInput")
    # bf16 consts: [:,0:128] causal tri mask, [:,128:256] ones, [256:640] zeros
    cb_d = nc.dram_tensor("cb", [128, 640], BF16, kind="ExternalInput")
    # bf16 ones row (lhsT of the reciprocal-broadcast matmul)
    cr_d = nc.dram_tensor("cr", [1, 64], BF16, kind="ExternalInput")
    y_d = nc.dram_tensor("y", [S, EMB], BF16, kind="ExternalOutput")

    rc = RECIP_APPROX_FAST_CONSTS

    with tile.TileContext(nc) as tc, nc.allow_low_precision(reason="bf16 attn"):
        with (
            tc.tile_pool(name="persist", bufs=1) as pp,
            tc.tile_pool(name="outp", bufs=4) as po,
            tc.tile_pool(name="attn", bufs=6) as pa,
            tc.tile_pool(name="recp", bufs=4) as prc,
            tc.tile_pool(name="ysb", bufs=3) as pyb,
            tc.tile_pool(name="ps_sc", bufs=2, space="PSUM") as ps_sc,
            tc.tile_pool(name="ps_out", bufs=2, space="PSUM") as ps_out,
            tc.tile_pool(name="ps_wv", bufs=2, space="PSUM") as ps_wv,
        ):
            WDT = FP8 if FP8_P2 else BF16
            xt = pp.tile([128, NEC, S], BF16, tag="xt")
            wq = pp.tile([128, 4, NEC, 128], WDT, tag="wq")
            wk = pp.tile([128, 4, NEC, 128], WDT, tag="wk")
            wv = pp.tile([128, NEC, GW], BF16, tag="wv")
            wr = pp.tile([128, 4, EMB], BF16, tag="wr")
            kt = pp.tile([128, NQC, S], BF16, tag="kt")
            qt = pp.tile([128, NQC, NQC, QW], BF16, tag="qt")  # [*, c, hp, q]
            # v rows padded to 128 so attn@V weight loads hit the fast-load
            # path (FWL needs 128 columns); rows 65-127 are never read back
            v = pp.tile([128, NSB, HPC, 128], BF16, tag="v")
            cb = pp.tile([128, 640], BF16, tag="cb")
            cr = pp.tile([1, 64], BF16, tag="cr")
            wz = pp.tile([128, 640], BF16, tag="wz")
            if FP8_P2:
                xt8 = pp.tile([128, NEC, S], FP8, tag="xt8")
            tri = cb[:, 0:128]

            # warmup tile is memset (no DMA dependency) so the PE can start
            # warming the HAM clock gate at t~1us, during the DMA-in window
            nc.vector.memset(wz[:], 0)
            for wi in range(12):
                wps = ps_sc.tile([128, 2, QW], F32, tag="sc", name=f"warm{wi}")
                nc.tensor.matmul(wps[:, 0, :], wz[:, 0:128], wz[:, 128:640],
                                 start=True, stop=True)
            # zero v's pad columns once (never read, but keeps NaNs out of
            # the attn@V matmul inputs); DVE is idle during the DMA window.
            # Emitted after the warmup matmuls so it can't delay their
            # dependency chain.
            nc.vector.memset(v[:, :, :, HD + 1:], 0)

            # DMA order: what the prefix needs first (wq/wk + x cols 0:512
            # for q/k chunk 0, then wv + the rest of x for the v projection);
            # wr (output proj) last.
            nc.sync.dma_start(cb[:], cb_d.ap())
            nc.sync.dma_start(cr[:], cr_d.ap())
            nc.sync.dma_start(wq[:], wq_d.ap())
            nc.sync.dma_start(wk[:], wk_d.ap())
            nc.sync.dma_start(wv[:], wv_d.ap())
            xt_src = xt_d.ap().rearrange("(e p) s -> p e s", p=128)
            # first seq block alone so p1(sb0) can start ~2us earlier
            nc.sync.dma_start(xt[:, :, 0:128], xt_src[:, :, 0:128])
            nc.sync.dma_start(xt[:, :, 128:QW], xt_src[:, :, 128:QW])
            for c in range(1, NQC):
                nc.sync.dma_start(xt[:, :, c * QW:(c + 1) * QW],
                                  xt_src[:, :, c * QW:(c + 1) * QW])
            nc.sync.dma_start(wr[:], wr_d.ap())
            # ones column of v (softmax denominator trick)
            nc.sync.dma_start(v[:, :, :, HD], cb_d.ap()[:, 128:256])

            # ---------------- weave units ----------------
            def p1_unit(sb, cp):
                ps = ps_wv.tile([128, GW], F32, tag="wv", name=f"p1_{sb}")
                for e in range(NEC):
                    nc.tensor.matmul(
                        ps[:], xt[:, e, sb * 128:(sb + 1) * 128], wv[:, e, :],
                        start=(e == 0), stop=(e == NEC - 1),
                    )
                cp(
                    v[:, sb, :, 0:HD],
                    ps[:].rearrange("p (h d) -> p h d", d=HD),
                )

            def p2_unit(c, hp, is_q, cp):
                w = wq if is_q else wk
                ps = ps_wv.tile([128, QW], F32, tag="wv",
                                name=f"p2_{c}_{hp}_{int(is_q)}")
                if FP8_P2:
                    # fp8 DoubleRow: two emb-chunks contracted per matmul
                    for t in range(NEC // 2):
                        nc.tensor.matmul(
                            ps[:], w[:, hp, 2 * t:2 * t + 2, :],
                            xt8[:, 2 * t:2 * t + 2, c * QW:(c + 1) * QW],
                            start=(t == 0), stop=(t == NEC // 2 - 1),
                            perf_mode=DR,
                        )
                else:
                    for e in range(NEC):
                        nc.tensor.matmul(
                            ps[:], w[:, hp, e, :],
                            xt[:, e, c * QW:(c + 1) * QW],
                            start=(e == 0), stop=(e == NEC - 1),
                        )
                if is_q:
                    cp(qt[:, c, hp, :], ps[:])
                else:
                    cp(kt[:, hp, c * QW:(c + 1) * QW], ps[:])

            def p4_unit(outtc, qc, sbl, cp, cp2=None):
                sb = qc * 4 + sbl
                ysb = pyb.tile([128, EMB], BF16, tag="ysb", name=f"ysb_{sb}")
                for ncol in range(2):
                    ps = ps_wv.tile([128, QW], F32, tag="wv",
                                    name=f"p4_{sb}_{ncol}")
                    for hp in range(4):
                        nc.tensor.matmul(
                            ps[:],
                            outtc[:, hp, sbl * 128:(sbl + 1) * 128],
                            wr[:, hp, ncol * QW:(ncol + 1) * QW],
                            start=(hp == 0), stop=(hp == 3),
                        )
                    (cp2 if (cp2 and ncol) else cp)(
                        ysb[:, ncol * QW:(ncol + 1) * QW], ps[:])
                    nc.sync.dma_start(
                        y_d.ap()[sb * 128:(sb + 1) * 128,
                                 ncol * QW:(ncol + 1) * QW],
                        ysb[:, ncol * QW:(ncol + 1) * QW])

            # ---------------- prefix: v sb0, q/k for (chunk 0, hp 0) ------
            # Only what attention(qc0, hp0, kb0) needs runs before the loop;
            # v sb1-3 are emitted inline between the first score iterations,
            # and q/k for hp1-3 of chunk 0 are preferred fillers (hpre), so
            # attention starts ~10us earlier.  (ScalarE is idle here, so
            # PSUM->SBUF copies go to it.)
            p1_unit(0, nc.scalar.copy)
            p2_unit(0, 0, True, nc.scalar.copy)
            p2_unit(0, 0, False, nc.scalar.copy)

            # Filler work woven into the attention loop so the PE never
            # stalls on ScalarE exp.  Whole units only: finer slicing was
            # tried and REGRESSED ~25-50us (interleaved accumulation groups
            # entangle the scheduler and stall the filler matmuls
            # themselves); the 2-deep attn pipeline absorbs whole-unit
            # bubbles instead.
            #   hpre: q/k for (chunk 0, hp>=1), force-finished at hp start.
            #   bulk: deadline-fenced projections (drain_due before chunk).
            #   p4q:  output projections, late filler.
            vec = nc.vector.tensor_copy
            # One deadline-ordered work queue, keyed (chunk, hp): each unit
            # is force-finished right before attention(qc, hp) first reads
            # it, but normally drips in earlier as filler.  Only hp0's q/k
            # plus the chunk's v blocks block at the chunk boundary, so the
            # chunk-start serial drain halves and ScalarE keeps getting
            # filler through the early chunk instead of idling.
            work = deque()
            for hp in range(1, 4):
                work.append(((0, hp), lambda hp=hp: p2_unit(0, hp, True, vec)))
                work.append(((0, hp), lambda hp=hp: p2_unit(0, hp, False, vec)))
            for c in (1, 2, 3):
                work.append(((c, 0), lambda c=c: p2_unit(c, 0, False, vec)))
                for sb in range(4 * c, 4 * c + 4):
                    work.append(((c, 0), lambda sb=sb: p1_unit(sb, vec)))
                work.append(((c, 0), lambda c=c: p2_unit(c, 0, True, vec)))
                for hp in range(1, 4):
                    work.append(((c, hp),
                                 lambda c=c, hp=hp: p2_unit(c, hp, True, vec)))
                    work.append(((c, hp),
                                 lambda c=c, hp=hp: p2_unit(c, hp, False, vec)))
            p4q = deque()

            def pop_filler(qc):
                if work:
                    work.popleft()[1]()
                elif p4q:
                    p4q.popleft()()

            def drain_hpre(qc, hp):
                while work and work[0][0] <= (qc, hp):
                    work.popleft()[1]()

            def drain_due(qc):
                drain_hpre(qc, 0)

            # ---------------- attention + output projection ----------------
            for qc in range(NQC):
                drain_due(qc)
                outtc = po.tile([128, NQC, QW], BF16, tag="outt",
                                name=f"outt_{qc}")
                for hp in range(4):
                    drain_hpre(qc, hp)
                    outps = [
                        ps_out.tile([128, QW], F32, tag="out",
                                    name=f"o_{qc}_{hp}_{s}")
                        for s in range(2)
                    ]
                    # kb order: diagonal blocks first, with the fully-masked
                    # left queries of diag block j (cols < 128j) skipped
                    # entirely (scores, exp, and attn@V all shrink).
                    kbs = ([(4 * qc + j, j * 128) for j in range(4)]
                           + [(kb, 0) for kb in range(4 * qc)])
                    n = len(kbs)
                    # Software-pipelined 2 deep: score matmuls run TWO
                    # iterations ahead of attn@V in the PE queue, so exp(kb)
                    # has ~2 iterations of runway on ScalarE and a whole
                    # filler unit can sit in the queue without starving it.
                    atq = deque()
                    for i in range(n + 2):
                        if i < n:
                            kb, off = kbs[i]
                            sc = ps_sc.tile([128, 2, QW], F32, tag="sc",
                                            name=f"sc_{qc}_{hp}_{kb}")
                            for s_ in range(2):
                                ho = s_ * HD
                                nc.tensor.matmul(
                                    sc[:, s_, off:],
                                    kt[ho:ho + HD, hp,
                                       kb * 128:(kb + 1) * 128],
                                    qt[ho:ho + HD, qc, hp, off:],
                                    start=True, stop=True,
                                )
                            at = pa.tile([128, 2, QW], BF16, tag="at",
                                         name=f"at_{qc}_{hp}_{kb}")
                            nc.scalar.activation(at[:, :, off:],
                                                 sc[:, :, off:], EXP,
                                                 scale=ESC)
                            if i < 4:  # diagonal block: causal mask
                                for s_ in range(2):
                                    nc.vector.tensor_mul(
                                        at[:, s_, off:off + 128],
                                        at[:, s_, off:off + 128],
                                        tri,
                                    )
                            atq.append((at, off))
                            if qc == 0 and hp == 0 and 1 <= i <= 3:
                                # v sb1-3 land just ahead of their attn@V
                                p1_unit(i, vec)
                        if i >= 2:
                            kbp = kbs[i - 2][0]
                            atp, offp = atq.popleft()
                            for s_ in range(2):
                                nc.tensor.matmul(
                                    outps[s_][:, offp:],
                                    v[:, kbp, 2 * hp + s_, :],
                                    atp[:, s_, offp:],
                                    start=(i == 2),
                                    stop=(i == n + 1),
                                )
                            if (i % 2 == 1) if work else (i % 4 == 3):
                                pop_filler(qc)

                    # epilogue: rows 0..63 = (attn@v).T numerator, row 64
                    # = softmax denominator.  Reciprocals for both heads run
                    # first; a filler chain then hides their DVE latency so
                    # the PE reaches the broadcast matmuls with inputs ready.
                    recs = []
                    for s_ in range(2):
                        den = prc.tile([1, QW], F32, tag="den",
                                       name=f"den_{qc}_{hp}_{s_}")
                        nc.vector.tensor_copy(den[0:1, :],
                                              outps[s_][HD:HD + 1, :])
                        rec = prc.tile([1, QW], BF16, tag="rec",
                                       name=f"rec_{qc}_{hp}_{s_}")
                        nc.vector._custom_dve(
                            RECIPROCAL_APPROX_FAST,
                            out=rec[0:1, :],
                            in0=den[0:1, :],
                            s0=rc["s0"], s1=rc["s1"], imm2=rc["imm2"],
                        )
                        recs.append(rec)
                    pop_filler(qc)
                    pop_filler(qc)
                    for s_ in range(2):
                        ho = s_ * HD
                        bct = ps_wv.tile([HD, QW], F32, tag="wv",
                                         name=f"bct_{qc}_{hp}_{s_}")
                        nc.tensor.matmul(bct[:], cr[0:1, 0:HD],
                                         recs[s_][0:1, :],
                                         start=True, stop=True)
                        bc = prc.tile([HD, QW], BF16, tag="bc",
                                      name=f"bc_{qc}_{hp}_{s_}")
                        nc.vector.tensor_copy(bc[:], bct[:])
                        nc.vector.tensor_mul(
                            outtc[ho:ho + HD, hp, :], outps[s_][0:HD, :],
                            bc[:],
                        )

                # P4 of this chunk becomes filler for later chunks
                for sbl in range(4):
                    p4q.append(
                        lambda outtc=outtc, qc=qc, sbl=sbl:
                            p4_unit(outtc, qc, sbl, nc.vector.tensor_copy))

            while work or p4q:
                pop_filler(NQC - 1)

    nc.compile()
    return nc


_NC_CACHE = None


def _get_nc():
    global _NC_CACHE
    if _NC_CACHE is None:
        _NC_CACHE = build()
    return _NC_CACHE


def make_in_maps(x, Wq, Wk, Wv, Wr):
    import ml_dtypes
    bf16 = ml_dtypes.bfloat16
    f8 = ml_dtypes.float8_e4m3
    wdt = f8 if FP8_P2 else bf16

    x = np.ascontiguousarray(x, dtype=np.float32)
    Wq = np.asarray(Wq, dtype=np.float32)
    Wk = np.asarray(Wk, dtype=np.float32)
    Wv = np.asarray(Wv, dtype=np.float32)
    Wr = np.asarray(Wr, dtype=np.float32)

    cb = np.zeros((128, 640), dtype=np.float32)
    cb[:, 0:128] = np.triu(np.ones((128, 128), dtype=np.float32))
    cb[:, 128:256] = 1.0
    # block-ones lhsT of the 2-head reciprocal-broadcast matmul:
    # row 0 -> out rows 0..63, row 1 -> out rows 64..127
    cb[0, 256:320] = 1.0
    cb[1, 320:384] = 1.0
    cb = cb.astype(bf16)
    cr = np.ones((1, 64), dtype=bf16)

    def swz(w):  # [1024, 512] -> [p, hp, e, n]
        return np.ascontiguousarray(
            w.reshape(NEC, 128, 4, 128).transpose(1, 2, 0, 3).astype(wdt))

    # q/k weights: fp8 path folds a x64 rescale in (fp8's sweet range) and
    # drops SCALE; both are undone by the exp() input scale ESC.
    qsc, ksc = (WSC, WSC) if FP8_P2 else (SCALE, 1.0)

    in_maps = []
    for core in range(NCORES):
        b, g = divmod(core, 2)
        hs = slice(g * GW, (g + 1) * GW)
        xtb = np.ascontiguousarray(x[b].T.astype(bf16))
        m = {
            "xt": xtb,
            "wq": swz(Wq[:, hs] * qsc),
            "wk": swz(Wk[:, hs] * ksc),
            "wv": np.ascontiguousarray(
                Wv[:, hs].reshape(NEC, 128, GW).transpose(1, 0, 2).astype(bf16)),
            "wr": np.ascontiguousarray(
                Wr[hs, :].reshape(4, 128, EMB).transpose(1, 0, 2).astype(bf16)),
            "cb": cb,
            "cr": cr,
        }
        if FP8_P2:
            m["xt8"] = np.ascontiguousarray(
                x[b].T.reshape(NEC, 128, S).transpose(1, 0, 2).astype(f8))
        in_maps.append(m)
    return in_maps


def kernel(x, Wq, Wk, Wv, Wr):
    in_maps = make_in_maps(x, Wq, Wk, Wv, Wr)
    nc = _get_nc()
    res = run_bass_kernel_spmd(nc, in_maps, core_ids=list(range(NCORES)))

    y = np.empty((B, S, EMB), dtype=np.float32)
    for b in range(B):
        y[b] = (res.results[2 * b]["y"].astype(np.float32)
                + res.results[2 * b + 1]["y"].astype(np.float32))
    return y



# revision 53
# speedup vs baseline: 1.0169x; 1.0169x over previous
"""Causal multi-head attention (B=4, S=2048, H=16, D=64, E=1024) on 8 TRN2 cores.

Sharding: data-parallel over batch (4) x tensor-parallel over heads (2 groups
of 8). Each core computes, for its (batch, head-group):
    q/k/v projections -> causal softmax attention -> output projection
and returns a partial [S, E] output (Wr row-split); the host adds the two
partials per batch.

Engine plan (per core):
  - Inputs (x, Wq, Wk, Wv, Wr) are host-cast to bf16: halves the DMA and
    runs the projection matmuls at 1 cycle/row.  qt/kt are bf16 too (f32r
    weights disable the fast weight-load path and ran ~2.3x slower).
  - Scores per head pair run as two row-tiled concurrent matmuls (head-even
    on PE rows 0-63, head-odd on rows 64-127, outputs in different PSUM
    banks), so the K=64 contraction doesn't waste half the array.
  - Diagonal key-blocks are processed FIRST with their fully-masked left
    query columns skipped entirely (scores, exp and attn@V all shrink by
    ~15%); only the 128-wide diagonal sub-block needs the tri mask.
  - exp() on ScalarE over one [128, 2, 512] ACTIVATE per key block (ACT has
    ~300ns fixed overhead per instruction - do not split it).
  - attn@V with a ones-column on V so the softmax denominators fall out of
    the same matmul (output row 64); V rows padded to 128 so its weight
    loads take the FWL path.  Denominators inverted with the fast
    custom-DVE reciprocal and broadcast to 64 rows via a tiny PE matmul.
  - The attention loop is software-pipelined 2 deep (scores run two
    iterations ahead of attn@V) and projection / output-projection work is
    woven in as WHOLE units of filler (finer slicing entangles the
    scheduler and regresses) so the PE never stalls on ScalarE exp and the
    HAM clock gate stays warm.
  - Startup: memset-fed warmup matmuls from t~1us, DMA ordered so the
    first chunk's q/k/v projections start as soon as their bytes land.
"""

from collections import deque

import numpy as np

import concourse.bacc as bacc
import concourse.bass as bass
import concourse.mybir as mybir
import concourse.tile as tile
from concourse.bass_utils import run_bass_kernel_spmd
from concourse.dve_ops import RECIP_APPROX_FAST_CONSTS, RECIPROCAL_APPROX_FAST

HEADS = 16
HD = 64
EMB = 1024
B, S = 4, 2048
SCALE = 1.0 / 8.0
NCORES = 8
HPC = HEADS // 2          # heads per core (8)
GW = HPC * HD             # head-group width (512)

F32 = mybir.dt.float32
F32R = mybir.dt.float32r
BF16 = mybir.dt.bfloat16
FP8 = mybir.dt.float8e4
DR = mybir.MatmulPerfMode.DoubleRow
EXP = mybir.ActivationFunctionType.Exp

FP8_P2 = False             # q/k projections in fp8e4m3 DoubleRow
WSC = 64.0                # host pre-scale on Wq/Wk so fp8 hits its sweet range
ESC = SCALE / (WSC * WSC) if FP8_P2 else 1.0  # folded into exp()'s input scale

NQC = 4                   # query chunks of 512
QW = 512                  # query chunk width
NEC = EMB // 128          # emb chunks (8)
NSB = S // 128            # seq blocks (16)


def build():
    nc = bacc.Bacc("TRN2", target_bir_lowering=False, debug=False)

    WDT = FP8 if FP8_P2 else BF16
    xt_d = nc.dram_tensor("xt", [EMB, S], BF16, kind="ExternalInput")
    # weights pre-swizzled on host to partition-major so every DMA is one
    # contiguous transfer
    wq_d = nc.dram_tensor("wq", [128, 4, NEC, 128], WDT, kind="ExternalInput")
    wk_d = nc.dram_tensor("wk", [128, 4, NEC, 128], WDT, kind="ExternalInput")
    wv_d = nc.dram_tensor("wv", [128, NEC, GW], BF16, kind="ExternalInput")
    wr_d = nc.dram_tensor("wr", [128, 4, EMB], BF16, kind="ExternalInput")
    if FP8_P2:
        xt8_d = nc.dram_tensor("xt8", [128, NEC, S], FP8, kind="ExternalInput")
    # bf16 consts: [:,0:128] causal tri mask, [:,128:256] ones, [256:640] zeros
    cb_d = nc.dram_tensor("cb", [128, 640], BF16, kind="ExternalInput")
    # bf16 ones row (lhsT of the reciprocal-broadcast matmul)
    cr_d = nc.dram_tensor("cr", [1, 64], BF16, kind="ExternalInput")
    y_d = nc.dram_tensor("y", [S, EMB], BF16, kind="ExternalOutput")

    rc = RECIP_APPROX_FAST_CONSTS

    with tile.TileContext(nc) as tc, nc.allow_low_precision(reason="bf16 attn"):
        with (
            tc.tile_pool(name="persist", bufs=1) as pp,
            tc.tile_pool(name="outp", bufs=4) as po,
            tc.tile_pool(name="attn", bufs=6) as pa,
            tc.tile_pool(name="recp", bufs=4) as prc,
            tc.tile_pool(name="ysb", bufs=3) as pyb,
            tc.tile_pool(name="ps_sc", bufs=2, space="PSUM") as ps_sc,
            tc.tile_pool(name="ps_out", bufs=2, space="PSUM") as ps_out,
            tc.tile_pool(name="ps_wv", bufs=2, space="PSUM") as ps_wv,
        ):
            WDT = FP8 if FP8_P2 else BF16
            xt = pp.tile([128, NEC, S], BF16, tag="xt")
            wq = pp.tile([128, 4, NEC, 128], WDT, tag="wq")
            wk = pp.tile([128, 4, NEC, 128], WDT, tag="wk")
            wv = pp.tile([128, NEC, GW], BF16, tag="wv")
            wr = pp.tile([128, 4, EMB], BF16, tag="wr")
            kt = pp.tile([128, NQC, S], BF16, tag="kt")
            qt = pp.tile([128, NQC, NQC, QW], BF16, tag="qt")  # [*, c, hp, q]
            # v rows padded to 128 so attn@V weight loads hit the fast-load
            # path (FWL needs 128 columns); rows 65-127 are never read back
            v = pp.tile([128, NSB, HPC, 128], BF16, tag="v")
            cb = pp.tile([128, 640], BF16, tag="cb")
            cr = pp.tile([1, 64], BF16, tag="cr")
            wz = pp.tile([128, 640], BF16, tag="wz")
            if FP8_P2:
                xt8 = pp.tile([128, NEC, S], FP8, tag="xt8")
            tri = cb[:, 0:128]

            # warmup tile is memset (no DMA dependency) so the PE can start
            # warming the HAM clock gate at t~1us, during the DMA-in window
            nc.vector.memset(wz[:], 0)
            # zero v's pad columns once (never read, but keeps NaNs out of
            # the attn@V matmul inputs); DVE is idle during the DMA window
            nc.vector.memset(v[:, :, :, HD + 1:], 0)
            for wi in range(20):
                wps = ps_sc.tile([128, 2, QW], F32, tag="sc", name=f"warm{wi}")
                nc.tensor.matmul(wps[:, 0, :], wz[:, 0:128], wz[:, 128:640],
                                 start=True, stop=True)

            # DMA order: exactly what the first score pipeline needs first
            # (wq/wk + x cols 0:512 for q/k of chunk 0), then the mask
            # consts and wv for the v projection; wr (output proj) last.
            nc.sync.dma_start(wq[:], wq_d.ap())
            nc.sync.dma_start(wk[:], wk_d.ap())
            xt_src = xt_d.ap().rearrange("(e p) s -> p e s", p=128)
            nc.sync.dma_start(xt[:, :, 0:128], xt_src[:, :, 0:128])
            nc.sync.dma_start(xt[:, :, 128:QW], xt_src[:, :, 128:QW])
            nc.sync.dma_start(cb[:], cb_d.ap())
            nc.sync.dma_start(cr[:], cr_d.ap())
            nc.sync.dma_start(wv[:], wv_d.ap())
            for c in range(1, NQC):
                nc.sync.dma_start(xt[:, :, c * QW:(c + 1) * QW],
                                  xt_src[:, :, c * QW:(c + 1) * QW])
            nc.sync.dma_start(wr[:], wr_d.ap())
            # ones column of v (softmax denominator trick)
            nc.sync.dma_start(v[:, :, :, HD], cb_d.ap()[:, 128:256])

            # ---------------- weave units ----------------
            def p1_unit(sb, cp):
                ps = ps_wv.tile([128, GW], F32, tag="wv", name=f"p1_{sb}")
                for e in range(NEC):
                    nc.tensor.matmul(
                        ps[:], xt[:, e, sb * 128:(sb + 1) * 128], wv[:, e, :],
                        start=(e == 0), stop=(e == NEC - 1),
                    )
                cp(
                    v[:, sb, :, 0:HD],
                    ps[:].rearrange("p (h d) -> p h d", d=HD),
                )

            def p2_unit(c, hp, is_q, cp):
                w = wq if is_q else wk
                ps = ps_wv.tile([128, QW], F32, tag="wv",
                                name=f"p2_{c}_{hp}_{int(is_q)}")
                if FP8_P2:
                    # fp8 DoubleRow: two emb-chunks contracted per matmul
                    for t in range(NEC // 2):
                        nc.tensor.matmul(
                            ps[:], w[:, hp, 2 * t:2 * t + 2, :],
                            xt8[:, 2 * t:2 * t + 2, c * QW:(c + 1) * QW],
                            start=(t == 0), stop=(t == NEC // 2 - 1),
                            perf_mode=DR,
                        )
                else:
                    for e in range(NEC):
                        nc.tensor.matmul(
                            ps[:], w[:, hp, e, :],
                            xt[:, e, c * QW:(c + 1) * QW],
                            start=(e == 0), stop=(e == NEC - 1),
                        )
                if is_q:
                    cp(qt[:, c, hp, :], ps[:])
                else:
                    cp(kt[:, hp, c * QW:(c + 1) * QW], ps[:])

            def p4_unit(outtc, qc, sbl, cp, cp2=None):
                sb = qc * 4 + sbl
                ysb = pyb.tile([128, EMB], BF16, tag="ysb", name=f"ysb_{sb}")
                for ncol in range(2):
                    ps = ps_wv.tile([128, QW], F32, tag="wv",
                                    name=f"p4_{sb}_{ncol}")
                    for hp in range(4):
                        nc.tensor.matmul(
                            ps[:],
                            outtc[:, hp, sbl * 128:(sbl + 1) * 128],
                            wr[:, hp, ncol * QW:(ncol + 1) * QW],
                            start=(hp == 0), stop=(hp == 3),
                        )
                    (cp2 if (cp2 and ncol) else cp)(
                        ysb[:, ncol * QW:(ncol + 1) * QW], ps[:])
                    nc.sync.dma_start(
                        y_d.ap()[sb * 128:(sb + 1) * 128,
                                 ncol * QW:(ncol + 1) * QW],
                        ysb[:, ncol * QW:(ncol + 1) * QW])

            # ---------------- prefix: v sb0, q/k for (chunk 0, hp 0) ------
            # Only what attention(qc0, hp0, kb0) needs runs before the loop;
            # v sb1-3 are emitted inline between the first score iterations,
            # and q/k for hp1-3 of chunk 0 are preferred fillers (hpre), so
            # attention starts ~10us earlier.  (ScalarE is idle here, so
            # PSUM->SBUF copies go to it.)
            # q/k first (their DMA lands first); wv streams in while they run
            p2_unit(0, 0, True, nc.scalar.copy)
            p2_unit(0, 0, False, nc.scalar.copy)
            p1_unit(0, nc.scalar.copy)

            # Filler work woven into the attention loop so the PE never
            # stalls on ScalarE exp.  Whole units only: finer slicing was
            # tried and REGRESSED ~25-50us (interleaved accumulation groups
            # entangle the scheduler and stall the filler matmuls
            # themselves); the 2-deep attn pipeline absorbs whole-unit
            # bubbles instead.
            #   hpre: q/k for (chunk 0, hp>=1), force-finished at hp start.
            #   bulk: deadline-fenced projections (drain_due before chunk).
            #   p4q:  output projections, late filler.
            vec = nc.vector.tensor_copy
            hpre = deque()
            for hp in range(1, 4):
                hpre.append((hp, lambda hp=hp: p2_unit(0, hp, True, vec)))
                hpre.append((hp, lambda hp=hp: p2_unit(0, hp, False, vec)))
            bulk = deque()
            for hp in range(4):
                bulk.append((1, lambda hp=hp: p2_unit(1, hp, False, vec)))
            for sb in range(4, 8):
                bulk.append((1, lambda sb=sb: p1_unit(sb, vec)))
            for hp in range(4):
                bulk.append((1, lambda hp=hp: p2_unit(1, hp, True, vec)))
            for c in (2, 3):
                for hp in range(4):
                    bulk.append((c, lambda c=c, hp=hp: p2_unit(
                        c, hp, False, vec)))
                for sb in range(4 * c, 4 * c + 4):
                    bulk.append((c, lambda sb=sb: p1_unit(sb, vec)))
                for hp in range(4):
                    bulk.append((c, lambda c=c, hp=hp: p2_unit(
                        c, hp, True, vec)))
            p4q = deque()

            def pop_filler(qc):
                if hpre:
                    hpre.popleft()[1]()
                elif bulk:
                    bulk.popleft()[1]()
                elif p4q:
                    p4q.popleft()()

            def drain_hpre(hp):
                while hpre and hpre[0][0] <= hp:
                    hpre.popleft()[1]()

            def drain_due(qc):
                while hpre:
                    hpre.popleft()[1]()
                while bulk and bulk[0][0] <= qc:
                    bulk.popleft()[1]()

            # ---------------- attention + output projection ----------------
            for qc in range(NQC):
                drain_due(qc)
                outtc = po.tile([128, NQC, QW], BF16, tag="outt",
                                name=f"outt_{qc}")
                for hp in range(4):
                    if qc == 0:
                        drain_hpre(hp)
                    outps = [
                        ps_out.tile([128, QW], F32, tag="out",
                                    name=f"o_{qc}_{hp}_{s}")
                        for s in range(2)
                    ]
                    # kb order: diagonal blocks first, with the fully-masked
                    # left queries of diag block j (cols < 128j) skipped
                    # entirely (scores, exp, and attn@V all shrink).
                    kbs = ([(4 * qc + j, j * 128) for j in range(4)]
                           + [(kb, 0) for kb in range(4 * qc)])
                    n = len(kbs)
                    # Software-pipelined 2 deep: score matmuls run TWO
                    # iterations ahead of attn@V in the PE queue, so exp(kb)
                    # has ~2 iterations of runway on ScalarE and a whole
                    # filler unit can sit in the queue without starving it.
                    atq = deque()
                    for i in range(n + 2):
                        if i < n:
                            kb, off = kbs[i]
                            sc = ps_sc.tile([128, 2, QW], F32, tag="sc",
                                            name=f"sc_{qc}_{hp}_{kb}")
                            for s_ in range(2):
                                ho = s_ * HD
                                nc.tensor.matmul(
                                    sc[:, s_, off:],
                                    kt[ho:ho + HD, hp,
                                       kb * 128:(kb + 1) * 128],
                                    qt[ho:ho + HD, qc, hp, off:],
                                    start=True, stop=True,
                                )
                            at = pa.tile([128, 2, QW], BF16, tag="at",
                                         name=f"at_{qc}_{hp}_{kb}")
                            nc.scalar.activation(at[:, :, off:],
                                                 sc[:, :, off:], EXP,
                                                 scale=ESC)
                            if i < 4:  # diagonal block: causal mask
                                for s_ in range(2):
                                    nc.vector.tensor_mul(
                                        at[:, s_, off:off + 128],
                                        at[:, s_, off:off + 128],
                                        tri,
                                    )
                            atq.append((at, off))
                            if qc == 0 and hp == 0 and 1 <= i <= 3:
                                # v sb1-3 land just ahead of their attn@V
                                p1_unit(i, vec)
                        if i >= 2:
                            kbp = kbs[i - 2][0]
                            atp, offp = atq.popleft()
                            for s_ in range(2):
                                nc.tensor.matmul(
                                    outps[s_][:, offp:],
                                    v[:, kbp, 2 * hp + s_, :],
                                    atp[:, s_, offp:],
                                    start=(i == 2),
                                    stop=(i == n + 1),
                                )
                            if (i % 2 == 1) if (hpre or bulk) else (i % 4 == 3):
                                pop_filler(qc)

                    # epilogue: rows 0..63 = (attn@v).T numerator, row 64
                    # = softmax denominator.  Reciprocals for both heads run
                    # first; a filler chain then hides their DVE latency so
                    # the PE reaches the broadcast matmuls with inputs ready.
                    recs = []
                    for s_ in range(2):
                        den = prc.tile([1, QW], F32, tag="den",
                                       name=f"den_{qc}_{hp}_{s_}")
                        nc.vector.tensor_copy(den[0:1, :],
                                              outps[s_][HD:HD + 1, :])
                        rec = prc.tile([1, QW], BF16, tag="rec",
                                       name=f"rec_{qc}_{hp}_{s_}")
                        nc.vector._custom_dve(
                            RECIPROCAL_APPROX_FAST,
                            out=rec[0:1, :],
                            in0=den[0:1, :],
                            s0=rc["s0"], s1=rc["s1"], imm2=rc["imm2"],
                        )
                        recs.append(rec)
                    pop_filler(qc)
                    pop_filler(qc)
                    for s_ in range(2):
                        ho = s_ * HD
                        bct = ps_wv.tile([HD, QW], F32, tag="wv",
                                         name=f"bct_{qc}_{hp}_{s_}")
                        nc.tensor.matmul(bct[:], cr[0:1, 0:HD],
                                         recs[s_][0:1, :],
                                         start=True, stop=True)
                        bc = prc.tile([HD, QW], BF16, tag="bc",
                                      name=f"bc_{qc}_{hp}_{s_}")
                        nc.vector.tensor_copy(bc[:], bct[:])
                        nc.vector.tensor_mul(
                            outtc[ho:ho + HD, hp, :], outps[s_][0:HD, :],
                            bc[:],
                        )

                # P4 of this chunk becomes filler for later chunks
                for sbl in range(4):
                    p4q.append(
                        lambda outtc=outtc, qc=qc, sbl=sbl:
                            p4_unit(outtc, qc, sbl, nc.vector.tensor_copy))

            while bulk or p4q:
                pop_filler(NQC - 1)

    nc.compile()
    return nc


_NC_CACHE = None


def _get_nc():
    global _NC_CACHE
    if _NC_CACHE is None:
        _NC_CACHE = build()
    return _NC_CACHE


def make_in_maps(x, Wq, Wk, Wv, Wr):
    import ml_dtypes
    bf16 = ml_dtypes.bfloat16
    f8 = ml_dtypes.float8_e4m3
    wdt = f8 if FP8_P2 else bf16

    x = np.ascontiguousarray(x, dtype=np.float32)
    Wq = np.asarray(Wq, dtype=np.float32)
    Wk = np.asarray(Wk, dtype=np.float32)
    Wv = np.asarray(Wv, dtype=np.float32)
    Wr = np.asarray(Wr, dtype=np.float32)

    cb = np.zeros((128, 640), dtype=np.float32)
    cb[:, 0:128] = np.triu(np.ones((128, 128), dtype=np.float32))
    cb[:, 128:256] = 1.0
    # block-ones lhsT of the 2-head reciprocal-broadcast matmul:
    # row 0 -> out rows 0..63, row 1 -> out rows 64..127
    cb[0, 256:320] = 1.0
    cb[1, 320:384] = 1.0
    cb = cb.astype(bf16)
    cr = np.ones((1, 64), dtype=bf16)

    def swz(w):  # [1024, 512] -> [p, hp, e, n]
        return np.ascontiguousarray(
            w.reshape(NEC, 128, 4, 128).transpose(1, 2, 0, 3).astype(wdt))

    # q/k weights: fp8 path folds a x64 rescale in (fp8's sweet range) and
    # drops SCALE; both are undone by the exp() input scale ESC.
    qsc, ksc = (WSC, WSC) if FP8_P2 else (SCALE, 1.0)

    in_maps = []
    for core in range(NCORES):
        b, g = divmod(core, 2)
        hs = slice(g * GW, (g + 1) * GW)
        xtb = np.ascontiguousarray(x[b].T.astype(bf16))
        m = {
            "xt": xtb,
            "wq": swz(Wq[:, hs] * qsc),
            "wk": swz(Wk[:, hs] * ksc),
            "wv": np.ascontiguousarray(
                Wv[:, hs].reshape(NEC, 128, GW).transpose(1, 0, 2).astype(bf16)),
            "wr": np.ascontiguousarray(
                Wr[hs, :].reshape(4, 128, EMB).transpose(1, 0, 2).astype(bf16)),
            "cb": cb,
            "cr": cr,
        }
        if FP8_P2:
            m["xt8"] = np.ascontiguousarray(
                x[b].T.reshape(NEC, 128, S).transpose(1, 0, 2).astype(f8))
        in_maps.append(m)
    return in_maps


def kernel(x, Wq, Wk, Wv, Wr):
    in_maps = make_in_maps(x, Wq, Wk, Wv, Wr)
    nc = _get_nc()
    res = run_bass_kernel_spmd(nc, in_maps, core_ids=list(range(NCORES)))

    y = np.empty((B, S, EMB), dtype=np.float32)
    for b in range(B):
        y[b] = (res.results[2 * b]["y"].astype(np.float32)
                + res.results[2 * b + 1]["y"].astype(np.float32))
    return y



# revision 56
# speedup vs baseline: 1.0272x; 1.0101x over previous
"""Causal multi-head attention (B=4, S=2048, H=16, D=64, E=1024) on 8 TRN2 cores.

Sharding: data-parallel over batch (4) x tensor-parallel over heads (2 groups
of 8). Each core computes, for its (batch, head-group):
    q/k/v projections -> causal softmax attention -> output projection
and returns a partial [S, E] output (Wr row-split); the host adds the two
partials per batch.

Engine plan (per core):
  - Inputs (x, Wq, Wk, Wv, Wr) are host-cast to bf16: halves the DMA and
    runs the projection matmuls at 1 cycle/row.  qt/kt are bf16 too (f32r
    weights disable the fast weight-load path and ran ~2.3x slower).
  - Scores per head pair run as two row-tiled concurrent matmuls (head-even
    on PE rows 0-63, head-odd on rows 64-127, outputs in different PSUM
    banks), so the K=64 contraction doesn't waste half the array.
  - Diagonal key-blocks are processed FIRST with their fully-masked left
    query columns skipped entirely (scores, exp and attn@V all shrink by
    ~15%); only the 128-wide diagonal sub-block needs the tri mask.
  - exp() on ScalarE over one [128, 2, 512] ACTIVATE per key block (ACT has
    ~300ns fixed overhead per instruction - do not split it).
  - attn@V with a ones-column on V so the softmax denominators fall out of
    the same matmul (output row 64); V rows padded to 128 so its weight
    loads take the FWL path.  Denominators inverted with the fast
    custom-DVE reciprocal and broadcast to 64 rows via a tiny PE matmul.
  - The attention loop is software-pipelined 2 deep (scores run two
    iterations ahead of attn@V) and projection / output-projection work is
    woven in as WHOLE units of filler (finer slicing entangles the
    scheduler and regresses) so the PE never stalls on ScalarE exp and the
    HAM clock gate stays warm.
  - Startup: memset-fed warmup matmuls from t~1us, DMA ordered so the
    first chunk's q/k/v projections start as soon as their bytes land.
"""

from collections import deque

import numpy as np

import concourse.bacc as bacc
import concourse.bass as bass
import concourse.mybir as mybir
import concourse.tile as tile
from concourse.bass_utils import run_bass_kernel_spmd
from concourse.dve_ops import RECIP_APPROX_FAST_CONSTS, RECIPROCAL_APPROX_FAST

HEADS = 16
HD = 64
EMB = 1024
B, S = 4, 2048
SCALE = 1.0 / 8.0
NCORES = 8
HPC = HEADS // 2          # heads per core (8)
GW = HPC * HD             # head-group width (512)

F32 = mybir.dt.float32
F32R = mybir.dt.float32r
BF16 = mybir.dt.bfloat16
FP8 = mybir.dt.float8e4
DR = mybir.MatmulPerfMode.DoubleRow
EXP = mybir.ActivationFunctionType.Exp

FP8_P2 = False             # q/k projections in fp8e4m3 DoubleRow
WSC = 64.0                # host pre-scale on Wq/Wk so fp8 hits its sweet range
ESC = SCALE / (WSC * WSC) if FP8_P2 else 1.0  # folded into exp()'s input scale

NQC = 4                   # query chunks of 512
QW = 512                  # query chunk width
NEC = EMB // 128          # emb chunks (8)
NSB = S // 128            # seq blocks (16)


def build():
    nc = bacc.Bacc("TRN2", target_bir_lowering=False, debug=False)

    WDT = FP8 if FP8_P2 else BF16
    xt_d = nc.dram_tensor("xt", [EMB, S], BF16, kind="ExternalInput")
    # weights pre-swizzled on host to partition-major so every DMA is one
    # contiguous transfer
    wq_d = nc.dram_tensor("wq", [128, 4, NEC, 128], WDT, kind="ExternalInput")
    wk_d = nc.dram_tensor("wk", [128, 4, NEC, 128], WDT, kind="ExternalInput")
    wv_d = nc.dram_tensor("wv", [128, NEC, GW], BF16, kind="ExternalInput")
    wr_d = nc.dram_tensor("wr", [128, 4, EMB], BF16, kind="ExternalInput")
    if FP8_P2:
        xt8_d = nc.dram_tensor("xt8", [128, NEC, S], FP8, kind="ExternalInput")
    # bf16 consts: [:,0:128] causal tri mask, [:,128:256] ones, [256:640] zeros
    cb_d = nc.dram_tensor("cb", [128, 640], BF16, kind="ExternalInput")
    # bf16 ones row (lhsT of the reciprocal-broadcast matmul)
    cr_d = nc.dram_tensor("cr", [1, 64], BF16, kind="ExternalInput")
    y_d = nc.dram_tensor("y", [S, EMB], BF16, kind="ExternalOutput")

    rc = RECIP_APPROX_FAST_CONSTS

    with tile.TileContext(nc) as tc, nc.allow_low_precision(reason="bf16 attn"):
        with (
            tc.tile_pool(name="persist", bufs=1) as pp,
            tc.tile_pool(name="outp", bufs=4) as po,
            tc.tile_pool(name="attn", bufs=8) as pa,
            tc.tile_pool(name="recp", bufs=4) as prc,
            tc.tile_pool(name="ysb", bufs=3) as pyb,
            tc.tile_pool(name="ps_sc", bufs=2, space="PSUM") as ps_sc,
            tc.tile_pool(name="ps_out", bufs=2, space="PSUM") as ps_out,
            tc.tile_pool(name="ps_wv", bufs=2, space="PSUM") as ps_wv,
        ):
            WDT = FP8 if FP8_P2 else BF16
            xt = pp.tile([128, NEC, S], BF16, tag="xt")
            wq = pp.tile([128, 4, NEC, 128], WDT, tag="wq")
            wk = pp.tile([128, 4, NEC, 128], WDT, tag="wk")
            wv = pp.tile([128, NEC, GW], BF16, tag="wv")
            wr = pp.tile([128, 4, EMB], BF16, tag="wr")
            kt = pp.tile([128, NQC, S], BF16, tag="kt")
            qt = pp.tile([128, NQC, NQC, QW], BF16, tag="qt")  # [*, c, hp, q]
            # v rows padded to 128 so attn@V weight loads hit the fast-load
            # path (FWL needs 128 columns); rows 65-127 are never read back
            v = pp.tile([128, NSB, HPC, 128], BF16, tag="v")
            cb = pp.tile([128, 640], BF16, tag="cb")
            cr = pp.tile([1, 64], BF16, tag="cr")
            wz = pp.tile([128, 640], BF16, tag="wz")
            if FP8_P2:
                xt8 = pp.tile([128, NEC, S], FP8, tag="xt8")
            tri = cb[:, 0:128]

            # warmup tile is memset (no DMA dependency) so the PE can start
            # warming the HAM clock gate at t~1us, during the DMA-in window
            nc.vector.memset(wz[:], 0)
            # zero v's pad columns once (never read, but keeps NaNs out of
            # the attn@V matmul inputs); DVE is idle during the DMA window
            nc.vector.memset(v[:, :, :, HD + 1:], 0)
            for wi in range(12):
                wps = ps_sc.tile([128, 2, QW], F32, tag="sc", name=f"warm{wi}")
                nc.tensor.matmul(wps[:, 0, :], wz[:, 0:128], wz[:, 128:640],
                                 start=True, stop=True)

            # DMA order: what the prefix needs first (wq/wk + x cols 0:512
            # for q/k chunk 0, then wv + the rest of x for the v projection);
            # wr (output proj) last.
            nc.sync.dma_start(cb[:], cb_d.ap())
            nc.sync.dma_start(cr[:], cr_d.ap())
            nc.sync.dma_start(wq[:], wq_d.ap())
            nc.sync.dma_start(wk[:], wk_d.ap())
            nc.sync.dma_start(wv[:], wv_d.ap())
            xt_src = xt_d.ap().rearrange("(e p) s -> p e s", p=128)
            # first seq block alone so p1(sb0) can start ~2us earlier
            nc.sync.dma_start(xt[:, :, 0:128], xt_src[:, :, 0:128])
            nc.sync.dma_start(xt[:, :, 128:QW], xt_src[:, :, 128:QW])
            for c in range(1, NQC):
                nc.sync.dma_start(xt[:, :, c * QW:(c + 1) * QW],
                                  xt_src[:, :, c * QW:(c + 1) * QW])
            nc.sync.dma_start(wr[:], wr_d.ap())
            # ones column of v (softmax denominator trick)
            nc.sync.dma_start(v[:, :, :, HD], cb_d.ap()[:, 128:256])

            # ---------------- weave units ----------------
            def p1_unit(sb, cp):
                ps = ps_wv.tile([128, GW], F32, tag="wv", name=f"p1_{sb}")
                for e in range(NEC):
                    nc.tensor.matmul(
                        ps[:], xt[:, e, sb * 128:(sb + 1) * 128], wv[:, e, :],
                        start=(e == 0), stop=(e == NEC - 1),
                    )
                cp(
                    v[:, sb, :, 0:HD],
                    ps[:].rearrange("p (h d) -> p h d", d=HD),
                )

            def p2_unit(c, hp, is_q, cp):
                w = wq if is_q else wk
                ps = ps_wv.tile([128, QW], F32, tag="wv",
                                name=f"p2_{c}_{hp}_{int(is_q)}")
                if FP8_P2:
                    # fp8 DoubleRow: two emb-chunks contracted per matmul
                    for t in range(NEC // 2):
                        nc.tensor.matmul(
                            ps[:], w[:, hp, 2 * t:2 * t + 2, :],
                            xt8[:, 2 * t:2 * t + 2, c * QW:(c + 1) * QW],
                            start=(t == 0), stop=(t == NEC // 2 - 1),
                            perf_mode=DR,
                        )
                else:
                    for e in range(NEC):
                        nc.tensor.matmul(
                            ps[:], w[:, hp, e, :],
                            xt[:, e, c * QW:(c + 1) * QW],
                            start=(e == 0), stop=(e == NEC - 1),
                        )
                if is_q:
                    cp(qt[:, c, hp, :], ps[:])
                else:
                    cp(kt[:, hp, c * QW:(c + 1) * QW], ps[:])

            def p4_unit(outtc, qc, sbl, cp, cp2=None):
                sb = qc * 4 + sbl
                ysb = pyb.tile([128, EMB], BF16, tag="ysb", name=f"ysb_{sb}")
                for ncol in range(2):
                    ps = ps_wv.tile([128, QW], F32, tag="wv",
                                    name=f"p4_{sb}_{ncol}")
                    for hp in range(4):
                        nc.tensor.matmul(
                            ps[:],
                            outtc[:, hp, sbl * 128:(sbl + 1) * 128],
                            wr[:, hp, ncol * QW:(ncol + 1) * QW],
                            start=(hp == 0), stop=(hp == 3),
                        )
                    (cp2 if (cp2 and ncol) else cp)(
                        ysb[:, ncol * QW:(ncol + 1) * QW], ps[:])
                    nc.sync.dma_start(
                        y_d.ap()[sb * 128:(sb + 1) * 128,
                                 ncol * QW:(ncol + 1) * QW],
                        ysb[:, ncol * QW:(ncol + 1) * QW])

            # ---------------- prefix: v sb0, q/k for (chunk 0, hp 0) ------
            # Only what attention(qc0, hp0, kb0) needs runs before the loop;
            # v sb1-3 are emitted inline between the first score iterations,
            # and q/k for hp1-3 of chunk 0 are preferred fillers (hpre), so
            # attention starts ~10us earlier.  (ScalarE is idle here, so
            # PSUM->SBUF copies go to it.)
            p1_unit(0, nc.scalar.copy)
            p2_unit(0, 0, True, nc.scalar.copy)
            p2_unit(0, 0, False, nc.scalar.copy)

            # Filler work woven into the attention loop so the PE never
            # stalls on ScalarE exp.  Whole units only: finer slicing was
            # tried and REGRESSED ~25-50us (interleaved accumulation groups
            # entangle the scheduler and stall the filler matmuls
            # themselves); the 2-deep attn pipeline absorbs whole-unit
            # bubbles instead.
            #   hpre: q/k for (chunk 0, hp>=1), force-finished at hp start.
            #   bulk: deadline-fenced projections (drain_due before chunk).
            #   p4q:  output projections, late filler.
            vec = nc.vector.tensor_copy
            hpre = deque()
            for hp in range(1, 4):
                hpre.append((hp, lambda hp=hp: p2_unit(0, hp, True, vec)))
                hpre.append((hp, lambda hp=hp: p2_unit(0, hp, False, vec)))
            bulk = deque()
            for hp in range(4):
                bulk.append((1, lambda hp=hp: p2_unit(1, hp, False, vec)))
            for sb in range(4, 8):
                bulk.append((1, lambda sb=sb: p1_unit(sb, vec)))
            for hp in range(4):
                bulk.append((1, lambda hp=hp: p2_unit(1, hp, True, vec)))
            for c in (2, 3):
                for hp in range(4):
                    bulk.append((c, lambda c=c, hp=hp: p2_unit(
                        c, hp, False, vec)))
                for sb in range(4 * c, 4 * c + 4):
                    bulk.append((c, lambda sb=sb: p1_unit(sb, vec)))
                for hp in range(4):
                    bulk.append((c, lambda c=c, hp=hp: p2_unit(
                        c, hp, True, vec)))
            p4q = deque()

            def pop_filler(qc):
                if hpre:
                    hpre.popleft()[1]()
                elif bulk:
                    bulk.popleft()[1]()
                elif p4q:
                    p4q.popleft()()

            def drain_hpre(hp):
                while hpre and hpre[0][0] <= hp:
                    hpre.popleft()[1]()

            def drain_due(qc):
                while hpre:
                    hpre.popleft()[1]()
                while bulk and bulk[0][0] <= qc:
                    bulk.popleft()[1]()

            # ---------------- attention + output projection ----------------
            for qc in range(NQC):
                drain_due(qc)
                outtc = po.tile([128, NQC, QW], BF16, tag="outt",
                                name=f"outt_{qc}")
                for hp in range(4):
                    if qc == 0:
                        drain_hpre(hp)
                    outps = [
                        ps_out.tile([128, QW], F32, tag="out",
                                    name=f"o_{qc}_{hp}_{s}")
                        for s in range(2)
                    ]
                    # kb order: diagonal blocks first, with the fully-masked
                    # left queries of diag block j (cols < 128j) skipped
                    # entirely (scores, exp, and attn@V all shrink).
                    kbs = ([(4 * qc + j, j * 128) for j in range(4)]
                           + [(kb, 0) for kb in range(4 * qc)])
                    n = len(kbs)
                    # Software-pipelined 2 deep: score matmuls run TWO
                    # iterations ahead of attn@V in the PE queue, so exp(kb)
                    # has ~2 iterations of runway on ScalarE and a whole
                    # filler unit can sit in the queue without starving it.
                    atq = deque()
                    for i in range(n + 2):
                        if i < n:
                            kb, off = kbs[i]
                            sc = ps_sc.tile([128, 2, QW], F32, tag="sc",
                                            name=f"sc_{qc}_{hp}_{kb}")
                            for s_ in range(2):
                                ho = s_ * HD
                                nc.tensor.matmul(
                                    sc[:, s_, off:],
                                    kt[ho:ho + HD, hp,
                                       kb * 128:(kb + 1) * 128],
                                    qt[ho:ho + HD, qc, hp, off:],
                                    start=True, stop=True,
                                )
                            at = pa.tile([128, 2, QW], BF16, tag="at",
                                         name=f"at_{qc}_{hp}_{kb}")
                            nc.scalar.activation(at[:, :, off:],
                                                 sc[:, :, off:], EXP,
                                                 scale=ESC)
                            if i < 4:  # diagonal block: causal mask
                                for s_ in range(2):
                                    nc.vector.tensor_mul(
                                        at[:, s_, off:off + 128],
                                        at[:, s_, off:off + 128],
                                        tri,
                                    )
                            atq.append((at, off))
                            if qc == 0 and hp == 0 and 1 <= i <= 3:
                                # v sb1-3 land just ahead of their attn@V
                                p1_unit(i, vec)
                        if i >= 2:
                            kbp = kbs[i - 2][0]
                            atp, offp = atq.popleft()
                            for s_ in range(2):
                                nc.tensor.matmul(
                                    outps[s_][:, offp:],
                                    v[:, kbp, 2 * hp + s_, :],
                                    atp[:, s_, offp:],
                                    start=(i == 2),
                                    stop=(i == n + 1),
                                )
                            # p4 filler is the only supply left in the last
                            # chunk: drip it 1-per-6 so it lasts the whole
                            # chunk instead of exhausting halfway
                            if (i % 2 == 1) if (hpre or bulk) else (i % 6 == 5):
                                pop_filler(qc)

                    # epilogue: rows 0..63 = (attn@v).T numerator, row 64
                    # = softmax denominator.  Reciprocals for both heads run
                    # first; a filler chain then hides their DVE latency so
                    # the PE reaches the broadcast matmuls with inputs ready.
                    recs = []
                    for s_ in range(2):
                        den = prc.tile([1, QW], F32, tag="den",
                                       name=f"den_{qc}_{hp}_{s_}")
                        nc.vector.tensor_copy(den[0:1, :],
                                              outps[s_][HD:HD + 1, :])
                        rec = prc.tile([1, QW], BF16, tag="rec",
                                       name=f"rec_{qc}_{hp}_{s_}")
                        nc.vector._custom_dve(
                            RECIPROCAL_APPROX_FAST,
                            out=rec[0:1, :],
                            in0=den[0:1, :],
                            s0=rc["s0"], s1=rc["s1"], imm2=rc["imm2"],
                        )
                        recs.append(rec)
                    pop_filler(qc)
                    pop_filler(qc)
                    for s_ in range(2):
                        ho = s_ * HD
                        bct = ps_wv.tile([HD, QW], F32, tag="wv",
                                         name=f"bct_{qc}_{hp}_{s_}")
                        nc.tensor.matmul(bct[:], cr[0:1, 0:HD],
                                         recs[s_][0:1, :],
                                         start=True, stop=True)
                        bc = prc.tile([HD, QW], BF16, tag="bc",
                                      name=f"bc_{qc}_{hp}_{s_}")
                        nc.vector.tensor_copy(bc[:], bct[:])
                        nc.vector.tensor_mul(
                            outtc[ho:ho + HD, hp, :], outps[s_][0:HD, :],
                            bc[:],
                        )

                # P4 of this chunk becomes filler for later chunks
                for sbl in range(4):
                    p4q.append(
                        lambda outtc=outtc, qc=qc, sbl=sbl:
                            p4_unit(outtc, qc, sbl, nc.vector.tensor_copy))

            while bulk or p4q:
                pop_filler(NQC - 1)

    nc.compile()
    return nc


_NC_CACHE = None


def _get_nc():
    global _NC_CACHE
    if _NC_CACHE is None:
        _NC_CACHE = build()
    return _NC_CACHE


def make_in_maps(x, Wq, Wk, Wv, Wr):
    import ml_dtypes
    bf16 = ml_dtypes.bfloat16
    f8 = ml_dtypes.float8_e4m3
    wdt = f8 if FP8_P2 else bf16

    x = np.ascontiguousarray(x, dtype=np.float32)
    Wq = np.asarray(Wq, dtype=np.float32)
    Wk = np.asarray(Wk, dtype=np.float32)
    Wv = np.asarray(Wv, dtype=np.float32)
    Wr = np.asarray(Wr, dtype=np.float32)

    cb = np.zeros((128, 640), dtype=np.float32)
    cb[:, 0:128] = np.triu(np.ones((128, 128), dtype=np.float32))
    cb[:, 128:256] = 1.0
    # block-ones lhsT of the 2-head reciprocal-broadcast matmul:
    # row 0 -> out rows 0..63, row 1 -> out rows 64..127
    cb[0, 256:320] = 1.0
    cb[1, 320:384] = 1.0
    cb = cb.astype(bf16)
    cr = np.ones((1, 64), dtype=bf16)

    def swz(w):  # [1024, 512] -> [p, hp, e, n]
        return np.ascontiguousarray(
            w.reshape(NEC, 128, 4, 128).transpose(1, 2, 0, 3).astype(wdt))

    # q/k weights: fp8 path folds a x64 rescale in (fp8's sweet range) and
    # drops SCALE; both are undone by the exp() input scale ESC.
    qsc, ksc = (WSC, WSC) if FP8_P2 else (SCALE, 1.0)

    in_maps = []
    for core in range(NCORES):
        b, g = divmod(core, 2)
        hs = slice(g * GW, (g + 1) * GW)
        xtb = np.ascontiguousarray(x[b].T.astype(bf16))
        m = {
            "xt": xtb,
            "wq": swz(Wq[:, hs] * qsc),
            "wk": swz(Wk[:, hs] * ksc),
            "wv": np.ascontiguousarray(
                Wv[:, hs].reshape(NEC, 128, GW).transpose(1, 0, 2).astype(bf16)),
            "wr": np.ascontiguousarray(
                Wr[hs, :].reshape(4, 128, EMB).transpose(1, 0, 2).astype(bf16)),
            "cb": cb,
            "cr": cr,
        }
        if FP8_P2:
            m["xt8"] = np.ascontiguousarray(
                x[b].T.reshape(NEC, 128, S).transpose(1, 0, 2).astype(f8))
        in_maps.append(m)
    return in_maps


def kernel(x, Wq, Wk, Wv, Wr):
    in_maps = make_in_maps(x, Wq, Wk, Wv, Wr)
    nc = _get_nc()
    res = run_bass_kernel_spmd(nc, in_maps, core_ids=list(range(NCORES)))

    y = np.empty((B, S, EMB), dtype=np.float32)
    for b in range(B):
        y[b] = (res.results[2 * b]["y"].astype(np.float32)
                + res.results[2 * b + 1]["y"].astype(np.float32))
    return y

